# revision 14
# baseline (speedup 1.0000x reference)
"""Trainium2 Bass kernel for nn_DecoderLayer (temporal self-attn decoder layer).

Sharding: data-parallel over batch B=8, one batch per NeuronCore. Weights are
replicated; each core runs an identical program on its batch slice.

Fully-fused single-pass design: one loop over 16 chunks (4 hw columns each,
CH=384 tokens). All intermediates stay in SBUF (no DRAM scratch). Activations
are feature-major X^T [D=256 -> 2 slabs x 128 partitions, tokens]; attention
works on per-head partition slices (heads = 32-partition blocks), causal mask
is folded into the scores via a PE bias-add matmul, softmax normalization is
folded into P before the AV matmuls. LayerNorm rstd uses exp(-0.5*ln(var)) so
the Activation engine only ever needs one table set (ln/exp/relu/square/copy).
"""
import numpy as np

D, NH, HD, FF = 256, 8, 32, 1024
B, T_OUT, T_IN, HW = 8, 96, 192, 64
CH = 384                 # tokens per chunk = 4 hw (x side), 2 hw (memory side)
NCH = (T_OUT * HW) // CH # 16
SCALE = float(1.0 / np.sqrt(HD))
LN_EPS_ARG = float(D) * float(D) * 1e-5   # bias for ln(D^2 * var)
LN_LOGD = float(np.log(float(D)))         # exp(-0.5*ln(arg) + ln(D)) = D/sqrt(arg)
MASKNEG = -100.0

_cached = {}


def _build():
    import concourse.bass as bass
    import concourse.mybir as mybir
    import concourse.tile as tile
    from concourse import bacc
    from concourse.masks import make_identity

    f32 = mybir.dt.float32
    f32r = mybir.dt.float32r
    bf16 = mybir.dt.bfloat16
    AF = mybir.ActivationFunctionType
    ALU = mybir.AluOpType

    nc = bacc.Bacc("TRN2", target_bir_lowering=False, debug=False)

    x_d = nc.dram_tensor("x", [T_OUT, HW, D], f32, kind="ExternalInput")
    mem_d = nc.dram_tensor("memory", [T_IN, HW, D], f32, kind="ExternalInput")
    out_d = nc.dram_tensor("out", [T_OUT, HW, D], f32, kind="ExternalOutput")

    WNAMES = [
        ("sa_wq", D, D), ("sa_wk", D, D), ("sa_wv", D, D), ("sa_wo", D, D),
        ("sa_ff_w1", D, FF), ("sa_ff_w2", FF, D),
        ("ca_wq", D, D), ("ca_wk", D, D), ("ca_wv", D, D), ("ca_wo", D, D),
        ("ff_w1", D, FF), ("ff_w2", FF, D),
    ]
    BNAMES = [("sa_bq", D), ("sa_bk", D), ("sa_bv", D), ("sa_bo", D),
              ("sa_ff_b1", FF), ("sa_ff_b2", D),
              ("ca_bq", D), ("ca_bk", D), ("ca_bv", D), ("ca_bo", D),
              ("ff_b1", FF), ("ff_b2", D),
              ("sa_ln1_g", D), ("sa_ln1_b", D), ("sa_ln2_g", D), ("sa_ln2_b", D),
              ("ln1_g", D), ("ln1_b", D), ("ln2_g", D), ("ln2_b", D),
              ("ln3_g", D), ("ln3_b", D)]
    wd = {n: nc.dram_tensor(n, [ki, ko], f32, kind="ExternalInput")
          for n, ki, ko in WNAMES}
    bd = {n: nc.dram_tensor(n, [k], f32, kind="ExternalInput") for n, k in BNAMES}

    with tile.TileContext(nc) as tc:
        _emit(nc, tc, bass, mybir, tile, make_identity, f32, f32r, bf16,
              AF, ALU, x_d, mem_d, out_d, wd, bd)
    nc.compile()
    return nc


def _emit(nc, tc, bass, mybir, tile, make_identity, f32, f32r, bf16,
          AF, ALU, x_d, mem_d, out_d, wd, bd):
    from contextlib import ExitStack

    es = ExitStack()
    cn = es.enter_context(tc.tile_pool(name="consts", bufs=1))
    pw = es.enter_context(tc.tile_pool(name="wts", bufs=1))
    pstg_cm = tc.tile_pool(name="wstage", bufs=1)
    pstg = pstg_cm.__enter__()
    ps = es.enter_context(tc.tile_pool(name="ps", bufs=2, space="PSUM"))

    # ---------------- constants ----------------
    idf = cn.tile([128, 128], f32, tag="idf")
    make_identity(nc, idf)
    idr = cn.tile([128, 128], f32r, tag="idr")     # f32r identity for transposes
    nc.gpsimd.tensor_copy(idr, idf)
    ones128b = cn.tile([128, 1], bf16, tag="ones128b")
    nc.vector.memset(ones128b, 1.0)
    ones96b = cn.tile([T_OUT, 1], bf16, tag="ones96b")
    nc.vector.memset(ones96b, 1.0)
    epsarg = cn.tile([1, 1], f32, tag="epsarg")
    nc.vector.memset(epsarg, LN_EPS_ARG)
    logd = cn.tile([1, 1], f32, tag="logd")
    nc.vector.memset(logd, LN_LOGD)
    # causal mask lhsT: L[p, m] = MASKNEG if m > p else 0  (bf16)
    maskf = cn.tile([T_OUT, T_OUT], f32, tag="maskf")
    nc.vector.memset(maskf, MASKNEG)
    nc.gpsimd.affine_select(out=maskf, in_=maskf, compare_op=ALU.is_ge,
                            fill=0.0, base=-1, pattern=[[1, T_OUT]],
                            channel_multiplier=-1)
    maskb = cn.tile([T_OUT, T_OUT], bf16, tag="maskb")
    nc.vector.tensor_copy(maskb, maskf)
    # identity replicated over the 4-head dim: [96, 4, 96] bf16 (materialized)
    idrep = cn.tile([T_OUT, 4, T_OUT], bf16, tag="idrep")
    idsrc = idf[0:T_OUT, 0:T_OUT]
    nc.gpsimd.tensor_copy(
        idrep, bass.AP(tensor=idsrc.tensor, offset=idsrc.offset,
                       ap=[idsrc.ap[0], [0, 4], idsrc.ap[1]]))

    # ---------------- weights ----------------
    def load_w(name, ki, ko, dtype, tag, scale=None):
        """[ki, ko] f32 DRAM -> [128, ki//128, ko] SBUF tile of dtype."""
        src = wd[name].ap().rearrange("(kt p) n -> p kt n", p=128)
        if dtype == f32r:
            wt = pw.tile([128, ki // 128, ko], f32r, tag=tag)
            nc.sync.dma_start(out=wt.bitcast(f32), in_=src)
            if scale is not None:
                nc.gpsimd.tensor_scalar(out=wt.bitcast(f32), in0=wt.bitcast(f32),
                                        scalar1=scale, scalar2=None, op0=ALU.mult)
            return wt
        stg = pstg.tile([128, ki // 128, ko], f32, tag=tag + "_stg")
        nc.sync.dma_start(out=stg, in_=src)
        wt = pw.tile([128, ki // 128, ko], bf16, tag=tag)
        if scale is not None:
            nc.gpsimd.tensor_scalar(out=wt, in0=stg, scalar1=scale,
                                    scalar2=None, op0=ALU.mult)
        else:
            nc.gpsimd.tensor_copy(wt, stg)
        return wt

    def load_b(name, k, tag, scale=None):
        bt = pw.tile([128, k // 128], f32, tag=tag)
        nc.sync.dma_start(out=bt, in_=bd[name].ap().rearrange(
            "(kt p) -> p kt", p=128))
        if scale is not None:
            nc.gpsimd.tensor_scalar(out=bt, in0=bt, scalar1=scale,
                                    scalar2=None, op0=ALU.mult)
        return bt

    wq = load_w("sa_wq", D, D, f32r, "wq", scale=SCALE)
    wk = load_w("sa_wk", D, D, f32r, "wk")
    wv = load_w("sa_wv", D, D, f32r, "wv")
    wo = load_w("sa_wo", D, D, bf16, "wo")
    w1 = load_w("sa_ff_w1", D, FF, bf16, "w1")
    w2 = load_w("sa_ff_w2", FF, D, bf16, "w2")
    cwq = load_w("ca_wq", D, D, bf16, "cwq", scale=SCALE)
    cwk = load_w("ca_wk", D, D, f32r, "cwk")
    cwv = load_w("ca_wv", D, D, f32r, "cwv")
    cwo = load_w("ca_wo", D, D, bf16, "cwo")
    fw1 = load_w("ff_w1", D, FF, bf16, "fw1")
    fw2 = load_w("ff_w2", FF, D, bf16, "fw2")

    bq = load_b("sa_bq", D, "bq", scale=SCALE)
    bk = load_b("sa_bk", D, "bk")
    fb1 = load_b("sa_ff_b1", FF, "fb1")
    fb2 = load_b("sa_ff_b2", D, "fb2")
    cbq = load_b("ca_bq", D, "cbq", scale=SCALE)
    cbk = load_b("ca_bk", D, "cbk")
    fbb1 = load_b("ff_b1", FF, "fbb1")
    fbb2 = load_b("ff_b2", D, "fbb2")

    # fused O-proj biases: bo2 = Wo^T bv + bo (per output feature)
    def oproj_bias(wox, bvname, boname, tag):
        bvf = load_b(bvname, D, tag + "v")
        bof = load_b(boname, D, tag + "o")
        bvr = pw.tile([128, 2, 128], bf16, tag=tag + "r")
        for kt in range(2):
            nc.vector.tensor_copy(bvr[:, kt, :],
                                  bvf[:, kt:kt + 1].to_broadcast([128, 128]))
        bo2 = pw.tile([128, 2], f32, tag=tag + "2")
        for mt in range(2):
            bps = ps.tile([128, 384], f32, tag="pp", bufs=2)
            for kt in range(2):
                nc.tensor.matmul(bps[:, 0:128], wox[:, kt, 128 * mt:128 * (mt + 1)],
                                 bvr[:, kt, :], start=(kt == 0), stop=(kt == 1))
            nc.vector.tensor_tensor(out=bo2[:, mt:mt + 1], in0=bps[:, 0:1],
                                    in1=bof[:, mt:mt + 1], op=ALU.add)
        return bo2

    bo2 = oproj_bias(wo, "sa_bv", "sa_bo", "bo")
    cbo2 = oproj_bias(cwo, "ca_bv", "ca_bo", "cbo")

    # layernorm params: g, b, and gD = -g/D
    def ln_params(gname, bname, tag):
        g = load_b(gname, D, tag + "g")
        b = load_b(bname, D, tag + "b")
        gD = pw.tile([128, 2], f32, tag=tag + "gD")
        nc.gpsimd.tensor_scalar(out=gD, in0=g, scalar1=-1.0 / D, scalar2=None,
                                op0=ALU.mult)
        return g, b, gD

    LNP = {k: ln_params(k + "_g", k + "_b", k) for k in
           ("sa_ln1", "sa_ln2", "ln1", "ln2", "ln3")}
    pstg_cm.__exit__(None, None, None)
    p = es.enter_context(tc.tile_pool(name="work", bufs=2))

    # ---------------- helpers ----------------
    def fm_layernorm(u, lnp, idx, tag, out_dtype=bf16):
        """u: [128, 2, CH] bf16 SBUF -> normalized tile (out_dtype).
        idx rotates the square op across engines."""
        g, b, gD = lnp
        usq = p.tile([128, 2, CH], bf16, tag="usq")
        nc.vector.tensor_tensor(out=usq, in0=u, in1=u, op=ALU.mult)
        ps_s = ps.tile([1, CH], f32, tag="row", bufs=2)
        ps_q = ps.tile([1, CH], f32, tag="row", bufs=2)
        for kt in range(2):
            nc.tensor.matmul(ps_s, ones128b, u[:, kt, :], start=(kt == 0),
                             stop=(kt == 1))
        for kt in range(2):
            nc.tensor.matmul(ps_q, ones128b, usq[:, kt, :], start=(kt == 0),
                             stop=(kt == 1))
        msq = p.tile([1, CH], f32, tag="lnmsq")
        nc.gpsimd.tensor_tensor(out=msq, in0=ps_s, in1=ps_s, op=ALU.mult)
        arg = p.tile([1, CH], f32, tag="lnarg")
        nc.gpsimd.scalar_tensor_tensor(out=arg, in0=ps_q, scalar=float(D),
                                       in1=msq, op0=ALU.mult, op1=ALU.subtract)
        lnt = p.tile([1, CH], f32, tag="lnt")
        nc.scalar.activation(out=lnt, in_=arg, func=AF.Ln, bias=epsarg)
        rc = p.tile([1, 2, CH], bf16, tag="lnrc")
        nc.scalar.activation(out=rc[:, 0, :], in_=lnt, func=AF.Exp,
                             scale=-0.5, bias=logd)
        nc.vector.tensor_tensor(out=rc[:, 1, :], in0=ps_s, in1=rc[:, 0, :],
                                op=ALU.mult)
        rcb = p.tile([128, 2, CH], bf16, tag="lnrcb")
        nc.gpsimd.partition_broadcast(
            rcb.rearrange("p a b -> p (a b)"), rc.rearrange("p a b -> p (a b)"))
        o = p.tile([128, 2, CH], out_dtype, tag="lno" + tag)
        for kt in range(2):
            ft = p.tile([128, CH], bf16, tag="lnf")
            nc.vector.tensor_scalar(out=ft, in0=rcb[:, 1, :],
                                    scalar1=gD[:, kt:kt + 1],
                                    scalar2=b[:, kt:kt + 1],
                                    op0=ALU.mult, op1=ALU.add)
            t1 = p.tile([128, CH], bf16, tag="lnt1")
            nc.vector.tensor_tensor(out=t1, in0=u[:, kt, :], in1=rcb[:, 0, :],
                                    op=ALU.mult)
            nc.vector.scalar_tensor_tensor(out=o[:, kt, :], in0=t1,
                                           scalar=g[:, kt:kt + 1], in1=ft,
                                           op0=ALU.mult, op1=ALU.add)
        return o

    def transpose_in(dst, dst_col, src, src_slice_fn, n, copy_engines):
        """PE-transpose n [96, 128] slabs of src into dst[:, kt, col...]."""
        for i in range(n):
            for kt in range(2):
                tp = ps.tile([128, 96], f32r, tag="tp", bufs=2)
                nc.tensor.transpose(tp, src_slice_fn(i, kt), idr[0:96, 0:96])
                eng = copy_engines[(i + kt) % len(copy_engines)]
                col = dst_col(i)
                eng(dst[:, kt, col:col + 96], tp)

    def pool_copy(dst, srctile):
        nc.gpsimd.tensor_copy(dst, srctile)

    def dve_copy(dst, srctile):
        nc.vector.tensor_copy(dst, srctile)

    # attention core for one (j, g) group of 4 heads
    def attn_group(qsrc, q0, ksrc, kslices, vparts, och, g, j, causal):
        """qsrc/ksrc: [128, 2, CH'] bf16 fm tiles; kslices: list of
        (col0, vtile) for each 96-token k block; och: [128, 2, CH] bf16."""
        nk = len(kslices)
        pts = []
        dn = ps.tile([1, 4 * T_OUT], f32, tag="row", bufs=2)
        for tt in range(nk):
            k0, _ = kslices[tt]
            st = ps.tile([T_OUT, 4, T_OUT], f32, tag="st", bufs=2)
            for hp in range(4):
                nc.tensor.matmul(st[:, hp, :],
                                 ksrc[32 * hp:32 * hp + 32, g, k0:k0 + 96],
                                 qsrc[32 * hp:32 * hp + 32, g, q0:q0 + 96],
                                 start=True, stop=not causal,
                                 skip_group_check=True,
                                 tile_position=(32 * hp, 0))
            if causal:
                nc.tensor.matmul(st, maskb, idrep, start=False, stop=True,
                                 skip_group_check=True)
            pt = p.tile([T_OUT, 4, T_OUT], bf16, tag=f"pt{tt}", bufs=3)
            nc.scalar.activation(out=pt, in_=st, func=AF.Exp)
            pts.append(pt)
        for tt in range(nk):
            nc.tensor.matmul(dn, ones96b, pts[tt].rearrange("p a b -> p (a b)"),
                             start=(tt == 0), stop=(tt == nk - 1))
        rec = p.tile([1, 4 * T_OUT], bf16, tag="rec")
        with nc.allow_low_precision(reason="softmax denom in bf16 is fine"):
            nc.vector.reciprocal(out=rec, in_=dn)
        recb = p.tile([T_OUT, 4, T_OUT], bf16, tag="recb")
        nc.gpsimd.partition_broadcast(recb.rearrange("p a b -> p (a b)"), rec)
        for tt in range(nk):
            nc.vector.tensor_tensor(out=pts[tt], in0=pts[tt], in1=recb,
                                    op=ALU.mult)
        avt = ps.tile([128, 96], f32r, tag="tp", bufs=2)
        av = avt.bitcast(f32)
        for hp in range(4):
            h = g * 4 + hp
            for tt in range(nk):
                _, vt = kslices[tt]
                nc.tensor.matmul(av[32 * hp:32 * hp + 32, :],
                                 vt[:, 32 * h:32 * h + 32], pts[tt][:, hp, :],
                                 start=(tt == 0), stop=(tt == nk - 1),
                                 skip_group_check=True,
                                 tile_position=(0, 32 * hp))
        nc.gpsimd.tensor_copy(och[:, g, 96 * j:96 * j + 96], av)

    def proj_fm(dst, w, bias, src, copy_fns):
        """dst[128, 2, CH'] (bf16) = w^T @ src + bias; src [128, 2, CH'] fm."""
        ncol = dst.shape[2]
        for mt in range(2):
            pp = ps.tile([128, 384], f32, tag="pp", bufs=2)
            for kt in range(2):
                nc.tensor.matmul(pp[:, 0:ncol], w[:, kt, 128 * mt:128 * (mt + 1)],
                                 src[:, kt, :], start=(kt == 0), stop=(kt == 1))
            copy_fns[mt % len(copy_fns)](dst[:, mt, :], pp[:, 0:ncol],
                                         bias[:, mt:mt + 1])
        return dst

    def act_copy_bias(dst, src, biasap):
        nc.scalar.activation(out=dst, in_=src, func=AF.Identity, bias=biasap)

    def dve_copy_bias(dst, src, biasap):
        nc.vector.tensor_scalar(out=dst, in0=src, scalar1=biasap,
                                scalar2=None, op0=ALU.add)

    def pool_copy_bias(dst, src, biasap):
        nc.gpsimd.tensor_scalar(out=dst, in0=src, scalar1=biasap,
                                scalar2=None, op0=ALU.add)

    # ---------------- main chunk loop ----------------
    for c in range(NCH):
        # ---- load + transpose x chunk: xch [128, 2, CH] f32r ----
        xin = p.tile([96, 4, D], f32r, tag="xin")
        nc.sync.dma_start(out=xin.bitcast(f32),
                          in_=x_d.ap()[:, 4 * c:4 * c + 4, :])
        xch = p.tile([128, 2, CH], f32r, tag="xch")
        transpose_in(xch, lambda i: 96 * i, xin,
                     lambda i, kt: xin[:, i, 128 * kt:128 * (kt + 1)],
                     4, [pool_copy, dve_copy])

        # ---- SA projections ----
        qch = p.tile([128, 2, CH], bf16, tag="qch")
        proj_fm(qch, wq, bq, xch, [act_copy_bias, pool_copy_bias])
        kch = p.tile([128, 2, CH], bf16, tag="kch")
        proj_fm(kch, wk, bk, xch, [dve_copy_bias, pool_copy_bias])
        vts = []
        for j in range(4):
            pv = ps.tile([128, 384], f32, tag="pp", bufs=2)
            for kt in range(2):
                nc.tensor.matmul(pv[0:96, 0:D], xch[:, kt, 96 * j:96 * j + 96],
                                 wv[:, kt, :], start=(kt == 0), stop=(kt == 1))
            vt = p.tile([96, D], bf16, tag="vt", bufs=8)
            nc.gpsimd.tensor_copy(vt, pv[0:96, 0:D])
            vts.append(vt)

        # ---- SA attention -> och ----
        och = p.tile([128, 2, CH], bf16, tag="och")
        for j in range(4):
            for g in range(2):
                attn_group(qch, 96 * j, kch, [(96 * j, vts[j])], None,
                           och, g, j, causal=True)

        # ---- SA O-proj + residual -> u ----
        u = p.tile([128, 2, CH], bf16, tag="u")
        for mt in range(2):
            pp = ps.tile([128, 384], f32, tag="pp", bufs=2)
            for kt in range(2):
                nc.tensor.matmul(pp, wo[:, kt, 128 * mt:128 * (mt + 1)],
                                 och[:, kt, :], start=(kt == 0), stop=(kt == 1))
            nc.vector.scalar_tensor_tensor(
                out=u[:, mt, :], in0=pp, scalar=bo2[:, mt:mt + 1],
                in1=xch[:, mt, :].bitcast(f32), op0=ALU.add, op1=ALU.add)
        h1 = fm_layernorm(u, LNP["sa_ln1"], 0, "h1")

        # ---- SA FFN ----
        hh = p.tile([128, FF // 128, CH], bf16, tag="hh")
        for mt in range(FF // 128):
            pp = ps.tile([128, 384], f32, tag="pp", bufs=2)
            for kt in range(2):
                nc.tensor.matmul(pp, w1[:, kt, 128 * mt:128 * (mt + 1)],
                                 h1[:, kt, :], start=(kt == 0), stop=(kt == 1))
            if mt % 2 == 0:
                nc.scalar.activation(out=hh[:, mt, :], in_=pp, func=AF.Relu,
                                     bias=fb1[:, mt:mt + 1])
            else:
                nc.gpsimd.tensor_scalar(out=hh[:, mt, :], in0=pp,
                                        scalar1=fb1[:, mt:mt + 1], scalar2=0.0,
                                        op0=ALU.add, op1=ALU.max)
        u2 = p.tile([128, 2, CH], bf16, tag="u2")
        for mt in range(2):
            pp = ps.tile([128, 384], f32, tag="pp", bufs=2)
            for kt in range(FF // 128):
                nc.tensor.matmul(pp, w2[:, kt, 128 * mt:128 * (mt + 1)],
                                 hh[:, kt, :], start=(kt == 0),
                                 stop=(kt == FF // 128 - 1))
            nc.vector.scalar_tensor_tensor(
                out=u2[:, mt, :], in0=pp, scalar=fb2[:, mt:mt + 1],
                in1=h1[:, mt, :], op0=ALU.add, op1=ALU.add)
        s2 = fm_layernorm(u2, LNP["sa_ln2"], 1, "s2")
        u3 = p.tile([128, 2, CH], bf16, tag="u3")
        nc.gpsimd.tensor_tensor(out=u3, in0=s2, in1=xch.bitcast(f32), op=ALU.add)
        x1 = fm_layernorm(u3, LNP["ln1"], 2, "x1")

        # ---- CA q-projection ----
        qc2 = p.tile([128, 2, CH], bf16, tag="qc2")
        proj_fm(qc2, cwq, cbq, x1, [act_copy_bias, dve_copy_bias])

        # ---- memory load/transpose + K/V projections (2 mchunks) ----
        kc2s, vcs = [], {}
        for mc2 in range(2):
            mch = p.tile([128, 2, CH], f32r, tag="mch", bufs=3)
            for tt in range(2):
                min_ = p.tile([96, 2, D], f32r, tag="min", bufs=4)
                nc.sync.dma_start(
                    out=min_.bitcast(f32),
                    in_=mem_d.ap()[96 * tt:96 * tt + 96,
                                   4 * c + 2 * mc2:4 * c + 2 * mc2 + 2, :])
                transpose_in(mch, lambda i, _tt=tt: 192 * i + 96 * _tt, min_,
                             lambda i, kt, _m=min_: _m[:, i, 128 * kt:128 * (kt + 1)],
                             2, [pool_copy, dve_copy])
            kc2 = p.tile([128, 2, CH], bf16, tag="kc2", bufs=3)
            proj_fm(kc2, cwk, cbk, mch, [dve_copy_bias, pool_copy_bias])
            kc2s.append(kc2)
            for jj in range(2):
                for tt in range(2):
                    seg = 192 * jj + 96 * tt
                    pv = ps.tile([128, 384], f32, tag="pp", bufs=2)
                    for kt in range(2):
                        nc.tensor.matmul(pv[0:96, 0:D],
                                         mch[:, kt, seg:seg + 96],
                                         cwv[:, kt, :], start=(kt == 0),
                                         stop=(kt == 1))
                    vc = p.tile([96, D], bf16, tag="vc", bufs=10)
                    nc.gpsimd.tensor_copy(vc, pv[0:96, 0:D])
                    vcs[(2 * mc2 + jj, tt)] = vc

        # ---- CA attention -> och2 ----
        och2 = p.tile([128, 2, CH], bf16, tag="och2")
        for j in range(4):
            kc2 = kc2s[j // 2]
            k0 = 192 * (j % 2)
            kslices = [(k0, vcs[(j, 0)]), (k0 + 96, vcs[(j, 1)])]
            for g in range(2):
                attn_group(qc2, 96 * j, kc2, kslices, None, och2, g, j,
                           causal=False)

        # ---- CA O-proj + residual ----
        u4 = p.tile([128, 2, CH], bf16, tag="u4")
        for mt in range(2):
            pp = ps.tile([128, 384], f32, tag="pp", bufs=2)
            for kt in range(2):
                nc.tensor.matmul(pp, cwo[:, kt, 128 * mt:128 * (mt + 1)],
                                 och2[:, kt, :], start=(kt == 0), stop=(kt == 1))
            nc.vector.scalar_tensor_tensor(
                out=u4[:, mt, :], in0=pp, scalar=cbo2[:, mt:mt + 1],
                in1=x1[:, mt, :], op0=ALU.add, op1=ALU.add)
        x2 = fm_layernorm(u4, LNP["ln2"], 3, "x2")

        # ---- decoder FFN ----
        hh2 = p.tile([128, FF // 128, CH], bf16, tag="hh2")
        for mt in range(FF // 128):
            pp = ps.tile([128, 384], f32, tag="pp", bufs=2)
            for kt in range(2):
                nc.tensor.matmul(pp, fw1[:, kt, 128 * mt:128 * (mt + 1)],
                                 x2[:, kt, :], start=(kt == 0), stop=(kt == 1))
            if mt % 2 == 0:
                nc.scalar.activation(out=hh2[:, mt, :], in_=pp, func=AF.Relu,
                                     bias=fbb1[:, mt:mt + 1])
            else:
                nc.gpsimd.tensor_scalar(out=hh2[:, mt, :], in0=pp,
                                        scalar1=fbb1[:, mt:mt + 1], scalar2=0.0,
                                        op0=ALU.add, op1=ALU.max)
        u5 = p.tile([128, 2, CH], bf16, tag="u5")
        for mt in range(2):
            pp = ps.tile([128, 384], f32, tag="pp", bufs=2)
            for kt in range(FF // 128):
                nc.tensor.matmul(pp, fw2[:, kt, 128 * mt:128 * (mt + 1)],
                                 hh2[:, kt, :], start=(kt == 0),
                                 stop=(kt == FF // 128 - 1))
            nc.vector.scalar_tensor_tensor(
                out=u5[:, mt, :], in0=pp, scalar=fbb2[:, mt:mt + 1],
                in1=x2[:, mt, :], op0=ALU.add, op1=ALU.add)
        oo = fm_layernorm(u5, LNP["ln3"], 4, "oo", out_dtype=f32r)

        # ---- transpose back + store ----
        tm = p.tile([96, 4, D], f32, tag="tm")
        for j in range(4):
            for kt in range(2):
                tq = ps.tile([128, 384], f32, tag="pp", bufs=2)
                tqv = tq[0:96, 0:128]
                nc.tensor.transpose(
                    tqv.bitcast(f32r),
                    oo[:, kt, 96 * j:96 * j + 96], idr)
                eng = pool_copy if (j + kt) % 2 == 0 else dve_copy
                eng(tm[:, j, 128 * kt:128 * (kt + 1)], tqv)
        nc.sync.dma_start(out=out_d.ap()[:, 4 * c:4 * c + 4, :], in_=tm)
    es.close()


def _make_runner(nc):
    """Cached jitted SPMD runner (avoids per-call retracing of
    run_bass_via_pjrt's fresh closures)."""
    import jax
    import numpy as np
    from jax.sharding import Mesh, PartitionSpec
    from jax.experimental.shard_map import shard_map
    import concourse.mybir as mybir
    from concourse.bass2jax import (_bass_exec_p, install_neuronx_cc_hook,
                                    partition_id_tensor)

    install_neuronx_cc_hook()
    partition_name = (nc.partition_id_tensor.name
                      if nc.partition_id_tensor else None)
    in_names, out_names, out_avals, zero_outs = [], [], [], []
    for alloc in nc.m.functions[0].allocations:
        if not isinstance(alloc, mybir.MemoryLocationSet):
            continue
        name = alloc.memorylocations[0].name
        if alloc.kind == "ExternalInput":
            if name != partition_name:
                in_names.append(name)
        elif alloc.kind == "ExternalOutput":
            shape = tuple(alloc.tensor_shape)
            dtype = mybir.dt.np(alloc.dtype)
            out_names.append(name)
            out_avals.append(jax.core.ShapedArray(shape, dtype))
            zero_outs.append(np.zeros(shape, dtype))
    n_params = len(in_names)
    all_names = list(in_names) + list(out_names)
    if partition_name is not None:
        all_names.append(partition_name)
    donate = tuple(range(n_params, n_params + len(out_names)))

    def _body(*args):
        operands = list(args)
        if partition_name is not None:
            operands.append(partition_id_tensor())
        return tuple(_bass_exec_p.bind(
            *operands, out_avals=tuple(out_avals), in_names=tuple(all_names),
            out_names=tuple(out_names), lowering_input_output_aliases=(),
            sim_require_finite=True, sim_require_nnan=True, nc=nc))

    devices = jax.devices()[:B]
    mesh = Mesh(np.asarray(devices), ("core",))
    in_specs = (PartitionSpec("core"),) * (n_params + len(out_names))
    out_specs = (PartitionSpec("core"),) * len(out_names)
    sharded = jax.jit(shard_map(_body, mesh=mesh, in_specs=in_specs,
                                out_specs=out_specs, check_rep=False),
                      donate_argnums=donate, keep_unused=True)

    def run(in_maps):
        concat_in = [np.concatenate([np.asarray(in_maps[c][nm])
                                     for c in range(B)], axis=0)
                     for nm in in_names]
        concat_zeros = [np.zeros((B * z.shape[0], *z.shape[1:]), z.dtype)
                        for z in zero_outs]
        out_arrs = sharded(*concat_in, *concat_zeros)
        oidx = out_names.index("out")
        a = np.asarray(out_arrs[oidx])
        return a.reshape(B, *out_avals[oidx].shape)

    return run


def kernel(**inputs):
    if "nc" not in _cached:
        _cached["nc"] = _build()
        _cached["run"] = _make_runner(_cached["nc"])
    inp = {k: np.asarray(v, dtype=np.float32) for k, v in inputs.items()}
    shared = {k: v for k, v in inp.items() if k not in ("x", "memory")}
    in_maps = []
    for c in range(B):
        m = dict(shared)
        m["x"] = np.ascontiguousarray(inp["x"][c])
        m["memory"] = np.ascontiguousarray(inp["memory"][c])
        in_maps.append(m)
    out = _cached["run"](in_maps)
    return np.ascontiguousarray(out).astype(np.float32)


# revision 17
# speedup vs baseline: 1.0528x; 1.0528x over previous
"""Trainium2 Bass kernel for nn_DecoderLayer (temporal self-attn decoder layer).

Sharding: data-parallel over batch B=8, one batch per NeuronCore. Weights are
replicated; each core runs an identical program on its batch slice.

Fully-fused single-pass design: one loop over 16 chunks (4 hw columns each,
CH=384 tokens). All intermediates stay in SBUF (no DRAM scratch). Activations
are feature-major X^T [D=256 -> 2 slabs x 128 partitions, tokens]; attention
works on per-head partition slices (heads = 32-partition blocks), causal mask
is folded into the scores via a PE bias-add matmul, softmax normalization is
folded into P before the AV matmuls. LayerNorm rstd uses exp(-0.5*ln(var)) so
the Activation engine only ever needs one table set (ln/exp/relu/square/copy).
"""
import numpy as np

D, NH, HD, FF = 256, 8, 32, 1024
B, T_OUT, T_IN, HW = 8, 96, 192, 64
CH = 384                 # tokens per chunk = 4 hw (x side), 2 hw (memory side)
NCH = (T_OUT * HW) // CH # 16
SCALE = float(1.0 / np.sqrt(HD))
LN_EPS_ARG = float(D) * float(D) * 1e-5   # bias for ln(D^2 * var)
LN_LOGD = float(np.log(float(D)))         # exp(-0.5*ln(arg) + ln(D)) = D/sqrt(arg)
MASKNEG = -100.0

_cached = {}


def _build():
    import concourse.bass as bass
    import concourse.mybir as mybir
    import concourse.tile as tile
    from concourse import bacc
    from concourse.masks import make_identity

    f32 = mybir.dt.float32
    f32r = mybir.dt.float32r
    bf16 = mybir.dt.bfloat16
    AF = mybir.ActivationFunctionType
    ALU = mybir.AluOpType

    nc = bacc.Bacc("TRN2", target_bir_lowering=False, debug=False)

    x_d = nc.dram_tensor("x", [T_OUT, HW, D], f32, kind="ExternalInput")
    mem_d = nc.dram_tensor("memory", [T_IN, HW, D], f32, kind="ExternalInput")
    out_d = nc.dram_tensor("out", [T_OUT, HW, D], f32, kind="ExternalOutput")

    WNAMES = [
        ("sa_wq", D, D), ("sa_wk", D, D), ("sa_wv", D, D), ("sa_wo", D, D),
        ("sa_ff_w1", D, FF), ("sa_ff_w2", FF, D),
        ("ca_wq", D, D), ("ca_wk", D, D), ("ca_wv", D, D), ("ca_wo", D, D),
        ("ff_w1", D, FF), ("ff_w2", FF, D),
    ]
    BNAMES = [("sa_bq", D), ("sa_bk", D), ("sa_bv", D), ("sa_bo", D),
              ("sa_ff_b1", FF), ("sa_ff_b2", D),
              ("ca_bq", D), ("ca_bk", D), ("ca_bv", D), ("ca_bo", D),
              ("ff_b1", FF), ("ff_b2", D),
              ("sa_ln1_g", D), ("sa_ln1_b", D), ("sa_ln2_g", D), ("sa_ln2_b", D),
              ("ln1_g", D), ("ln1_b", D), ("ln2_g", D), ("ln2_b", D),
              ("ln3_g", D), ("ln3_b", D)]
    wd = {n: nc.dram_tensor(n, [ki, ko], f32, kind="ExternalInput")
          for n, ki, ko in WNAMES}
    bd = {n: nc.dram_tensor(n, [k], f32, kind="ExternalInput") for n, k in BNAMES}

    with tile.TileContext(nc) as tc:
        _emit(nc, tc, bass, mybir, tile, make_identity, f32, f32r, bf16,
              AF, ALU, x_d, mem_d, out_d, wd, bd)
    nc.compile()
    return nc


def _emit(nc, tc, bass, mybir, tile, make_identity, f32, f32r, bf16,
          AF, ALU, x_d, mem_d, out_d, wd, bd):
    from contextlib import ExitStack

    es = ExitStack()
    cn = es.enter_context(tc.tile_pool(name="consts", bufs=1))
    pw = es.enter_context(tc.tile_pool(name="wts", bufs=1))
    pstg_cm = tc.tile_pool(name="wstage", bufs=1)
    pstg = pstg_cm.__enter__()
    ps = es.enter_context(tc.tile_pool(name="ps", bufs=2, space="PSUM"))

    # ---------------- constants ----------------
    idf = cn.tile([128, 128], f32, tag="idf")
    make_identity(nc, idf)
    idr = cn.tile([128, 128], f32r, tag="idr")     # f32r identity for transposes
    nc.gpsimd.tensor_copy(idr, idf)
    ones128b = cn.tile([128, 1], bf16, tag="ones128b")
    nc.vector.memset(ones128b, 1.0)
    ones96b = cn.tile([T_OUT, 1], bf16, tag="ones96b")
    nc.vector.memset(ones96b, 1.0)
    epsarg = cn.tile([1, 1], f32, tag="epsarg")
    nc.vector.memset(epsarg, LN_EPS_ARG)
    logd = cn.tile([1, 1], f32, tag="logd")
    nc.vector.memset(logd, LN_LOGD)
    # causal mask lhsT: L[p, m] = MASKNEG if m > p else 0  (bf16)
    maskf = cn.tile([T_OUT, T_OUT], f32, tag="maskf")
    nc.vector.memset(maskf, MASKNEG)
    nc.gpsimd.affine_select(out=maskf, in_=maskf, compare_op=ALU.is_ge,
                            fill=0.0, base=-1, pattern=[[1, T_OUT]],
                            channel_multiplier=-1)
    maskb = cn.tile([T_OUT, T_OUT], bf16, tag="maskb")
    nc.vector.tensor_copy(maskb, maskf)
    # identity replicated over the 4-head dim: [96, 4, 96] bf16 (materialized)
    idrep = cn.tile([T_OUT, 4, T_OUT], bf16, tag="idrep")
    idsrc = idf[0:T_OUT, 0:T_OUT]
    nc.gpsimd.tensor_copy(
        idrep, bass.AP(tensor=idsrc.tensor, offset=idsrc.offset,
                       ap=[idsrc.ap[0], [0, 4], idsrc.ap[1]]))
    # head-block select: blkb[k, p] = 1 iff k == 32*(p//32)
    blkb = cn.tile([128, 128], bf16, tag="blkb")
    nc.vector.memset(blkb, 0.0)
    for a in range(4):
        nc.vector.memset(blkb[32 * a:32 * a + 1, 32 * a:32 * a + 32], 1.0)
    ones96x128 = cn.tile([T_OUT, 128], bf16, tag="ones96x128")
    nc.vector.memset(ones96x128, 1.0)

    # ---------------- weights ----------------
    def load_w(name, ki, ko, dtype, tag, scale=None):
        """[ki, ko] f32 DRAM -> [128, ki//128, ko] SBUF tile of dtype."""
        src = wd[name].ap().rearrange("(kt p) n -> p kt n", p=128)
        if dtype == f32r:
            wt = pw.tile([128, ki // 128, ko], f32r, tag=tag)
            nc.sync.dma_start(out=wt.bitcast(f32), in_=src)
            if scale is not None:
                nc.gpsimd.tensor_scalar(out=wt.bitcast(f32), in0=wt.bitcast(f32),
                                        scalar1=scale, scalar2=None, op0=ALU.mult)
            return wt
        stg = pstg.tile([128, ki // 128, ko], f32, tag=tag + "_stg")
        nc.sync.dma_start(out=stg, in_=src)
        wt = pw.tile([128, ki // 128, ko], bf16, tag=tag)
        if scale is not None:
            nc.gpsimd.tensor_scalar(out=wt, in0=stg, scalar1=scale,
                                    scalar2=None, op0=ALU.mult)
        else:
            nc.gpsimd.tensor_copy(wt, stg)
        return wt

    def load_b(name, k, tag, scale=None):
        bt = pw.tile([128, k // 128], f32, tag=tag)
        nc.sync.dma_start(out=bt, in_=bd[name].ap().rearrange(
            "(kt p) -> p kt", p=128))
        if scale is not None:
            nc.gpsimd.tensor_scalar(out=bt, in0=bt, scalar1=scale,
                                    scalar2=None, op0=ALU.mult)
        return bt

    wq = load_w("sa_wq", D, D, f32r, "wq", scale=SCALE)
    wk = load_w("sa_wk", D, D, f32r, "wk")
    wv = load_w("sa_wv", D, D, f32r, "wv")
    wo = load_w("sa_wo", D, D, bf16, "wo")
    w1 = load_w("sa_ff_w1", D, FF, bf16, "w1")
    w2 = load_w("sa_ff_w2", FF, D, bf16, "w2")
    cwq = load_w("ca_wq", D, D, bf16, "cwq", scale=SCALE)
    cwk = load_w("ca_wk", D, D, f32r, "cwk")
    cwv = load_w("ca_wv", D, D, f32r, "cwv")
    cwo = load_w("ca_wo", D, D, bf16, "cwo")
    fw1 = load_w("ff_w1", D, FF, bf16, "fw1")
    fw2 = load_w("ff_w2", FF, D, bf16, "fw2")

    bq = load_b("sa_bq", D, "bq", scale=SCALE)
    bk = load_b("sa_bk", D, "bk")
    fb1 = load_b("sa_ff_b1", FF, "fb1")
    fb2 = load_b("sa_ff_b2", D, "fb2")
    cbq = load_b("ca_bq", D, "cbq", scale=SCALE)
    cbk = load_b("ca_bk", D, "cbk")
    fbb1 = load_b("ff_b1", FF, "fbb1")
    fbb2 = load_b("ff_b2", D, "fbb2")

    # fused O-proj biases: bo2 = Wo^T bv + bo (per output feature)
    def oproj_bias(wox, bvname, boname, tag):
        bvf = load_b(bvname, D, tag + "v")
        bof = load_b(boname, D, tag + "o")
        bvr = pw.tile([128, 2, 128], bf16, tag=tag + "r")
        for kt in range(2):
            nc.vector.tensor_copy(bvr[:, kt, :],
                                  bvf[:, kt:kt + 1].to_broadcast([128, 128]))
        bo2 = pw.tile([128, 2], f32, tag=tag + "2")
        for mt in range(2):
            bps = ps.tile([128, 384], f32, tag="pp", bufs=2)
            for kt in range(2):
                nc.tensor.matmul(bps[:, 0:128], wox[:, kt, 128 * mt:128 * (mt + 1)],
                                 bvr[:, kt, :], start=(kt == 0), stop=(kt == 1))
            nc.vector.tensor_tensor(out=bo2[:, mt:mt + 1], in0=bps[:, 0:1],
                                    in1=bof[:, mt:mt + 1], op=ALU.add)
        return bo2

    bo2 = oproj_bias(wo, "sa_bv", "sa_bo", "bo")
    cbo2 = oproj_bias(cwo, "ca_bv", "ca_bo", "cbo")

    # layernorm params: g, b, and gD = -g/D
    def ln_params(gname, bname, tag):
        g = load_b(gname, D, tag + "g")
        b = load_b(bname, D, tag + "b")
        gD = pw.tile([128, 2], f32, tag=tag + "gD")
        nc.gpsimd.tensor_scalar(out=gD, in0=g, scalar1=-1.0 / D, scalar2=None,
                                op0=ALU.mult)
        return g, b, gD

    LNP = {k: ln_params(k + "_g", k + "_b", k) for k in
           ("sa_ln1", "sa_ln2", "ln1", "ln2", "ln3")}
    pstg_cm.__exit__(None, None, None)
    p = es.enter_context(tc.tile_pool(name="work", bufs=2))

    # ---------------- helpers ----------------
    def fm_layernorm(u, lnp, idx, tag, out_dtype=bf16):
        """u: [128, 2, CH] bf16 SBUF -> normalized tile (out_dtype)."""
        g, b, gD = lnp
        usq = p.tile([128, 2, CH], bf16, tag="usq")
        nc.scalar.activation(out=usq, in_=u, func=AF.Square)
        ps_s = ps.tile([1, CH], f32, tag="row", bufs=2)
        ps_q = ps.tile([1, CH], f32, tag="row", bufs=2)
        for kt in range(2):
            nc.tensor.matmul(ps_s, ones128b, u[:, kt, :], start=(kt == 0),
                             stop=(kt == 1))
        for kt in range(2):
            nc.tensor.matmul(ps_q, ones128b, usq[:, kt, :], start=(kt == 0),
                             stop=(kt == 1))
        msq = p.tile([1, CH], f32, tag="lnmsq")
        nc.gpsimd.tensor_tensor(out=msq, in0=ps_s, in1=ps_s, op=ALU.mult)
        arg = p.tile([1, CH], f32, tag="lnarg")
        nc.gpsimd.scalar_tensor_tensor(out=arg, in0=ps_q, scalar=float(D),
                                       in1=msq, op0=ALU.mult, op1=ALU.subtract)
        # rstd = D * arg^-0.5 = 1/sqrt(var)
        rc = p.tile([1, 2, CH], bf16, tag="lnrc")
        with nc.allow_low_precision(reason="rstd in bf16 is fine"):
            nc.vector.tensor_scalar(out=rc[:, 0, :], in0=arg, scalar1=-0.5,
                                    scalar2=float(D), op0=ALU.pow, op1=ALU.mult)
        nc.gpsimd.tensor_tensor(out=rc[:, 1, :], in0=ps_s, in1=rc[:, 0, :],
                                op=ALU.mult)
        rcb = p.tile([128, 2, CH], bf16, tag="lnrcb")
        nc.gpsimd.partition_broadcast(
            rcb.rearrange("p a b -> p (a b)"), rc.rearrange("p a b -> p (a b)"))
        o = p.tile([128, 2, CH], out_dtype, tag="lno" + tag)
        for kt in range(2):
            ft = p.tile([128, CH], bf16, tag="lnf")
            nc.vector.tensor_scalar(out=ft, in0=rcb[:, 1, :],
                                    scalar1=gD[:, kt:kt + 1],
                                    scalar2=b[:, kt:kt + 1],
                                    op0=ALU.mult, op1=ALU.add)
            t1 = p.tile([128, CH], bf16, tag="lnt1")
            if kt == 0:
                nc.vector.tensor_tensor(out=t1, in0=u[:, kt, :],
                                        in1=rcb[:, 0, :], op=ALU.mult)
                nc.vector.scalar_tensor_tensor(out=o[:, kt, :], in0=t1,
                                               scalar=g[:, kt:kt + 1], in1=ft,
                                               op0=ALU.mult, op1=ALU.add)
            else:
                nc.gpsimd.tensor_tensor(out=t1, in0=u[:, kt, :],
                                        in1=rcb[:, 0, :], op=ALU.mult)
                nc.gpsimd.scalar_tensor_tensor(out=o[:, kt, :], in0=t1,
                                               scalar=g[:, kt:kt + 1], in1=ft,
                                               op0=ALU.mult, op1=ALU.add)
        return o

    def transpose_in(dst, dst_col, src, src_slice_fn, n, copy_engines):
        """PE-transpose n [96, 128] slabs of src into dst[:, kt, col...]."""
        for i in range(n):
            for kt in range(2):
                tp = ps.tile([128, 96], f32r, tag="tp", bufs=2)
                nc.tensor.transpose(tp, src_slice_fn(i, kt), idr[0:96, 0:96])
                eng = copy_engines[(i + kt) % len(copy_engines)]
                col = dst_col(i)
                eng(dst[:, kt, col:col + 96], tp)

    def pool_copy(dst, srctile):
        nc.gpsimd.tensor_copy(dst, srctile)

    def dve_copy(dst, srctile):
        nc.vector.tensor_copy(dst, srctile)

    # attention core for one (j, g) group of 4 heads
    def attn_group(qsrc, q0, ksrc, kslices, vparts, och, g, j, causal):
        """qsrc/ksrc: [128, 2, CH'] bf16 fm tiles; kslices: list of
        (col0, vtile) for each 96-token k block; och: [128, 2, CH] bf16."""
        nk = len(kslices)
        pts = []
        for tt in range(nk):
            k0, _ = kslices[tt]
            st = ps.tile([T_OUT, 4, T_OUT], f32, tag="st", bufs=2)
            for hp in range(4):
                nc.tensor.matmul(st[:, hp, :],
                                 ksrc[32 * hp:32 * hp + 32, g, k0:k0 + 96],
                                 qsrc[32 * hp:32 * hp + 32, g, q0:q0 + 96],
                                 start=True, stop=not causal,
                                 skip_group_check=True,
                                 tile_position=(32 * hp, 0))
            if causal:
                nc.tensor.matmul(st, maskb, idrep, start=False, stop=True,
                                 skip_group_check=True)
            pt = p.tile([T_OUT, 4, T_OUT], bf16, tag=f"pt{tt}", bufs=3)
            nc.scalar.activation(out=pt, in_=st, func=AF.Exp)
            pts.append(pt)
        # per-head denominators as 1-row stripes of a [128, 96] psum tile;
        # a first full-tile matmul fills the unused partitions with benign
        # positive values so the whole-tile reciprocal stays finite.
        dnvt = ps.tile([128, 96], f32r, tag="tp", bufs=2)
        dnv = dnvt.bitcast(f32)
        nc.tensor.matmul(dnv, ones96x128, pts[0][:, 0, :], start=True,
                         stop=False, skip_group_check=True)
        for hp in range(4):
            for tt in range(nk):
                nc.tensor.matmul(dnv[32 * hp:32 * hp + 1, :], ones96b,
                                 pts[tt][:, hp, :], start=(tt == 0),
                                 stop=(tt == nk - 1), skip_group_check=True,
                                 tile_position=(0, 32 * hp))
        recn = p.tile([128, 96], bf16, tag="recn", bufs=3)
        with nc.allow_low_precision(reason="softmax denom in bf16 is fine"):
            nc.vector.reciprocal(out=recn, in_=dnv)
        recbt = ps.tile([128, 96], f32r, tag="tp", bufs=2)
        recb = recbt.bitcast(f32)
        nc.tensor.matmul(recb, blkb, recn, start=True, stop=True,
                         skip_group_check=True)
        avt = ps.tile([128, 96], f32r, tag="tp", bufs=2)
        av = avt.bitcast(f32)
        for hp in range(4):
            h = g * 4 + hp
            for tt in range(nk):
                _, vt = kslices[tt]
                nc.tensor.matmul(av[32 * hp:32 * hp + 32, :],
                                 vt[:, 32 * h:32 * h + 32], pts[tt][:, hp, :],
                                 start=(tt == 0), stop=(tt == nk - 1),
                                 skip_group_check=True,
                                 tile_position=(0, 32 * hp))
        if (j + g) % 2 == 0:
            nc.gpsimd.tensor_tensor(out=och[:, g, 96 * j:96 * j + 96],
                                    in0=av, in1=recb, op=ALU.mult)
        else:
            nc.vector.tensor_tensor(out=och[:, g, 96 * j:96 * j + 96],
                                    in0=av, in1=recb, op=ALU.mult)

    def proj_fm(dst, w, bias, src, copy_fns):
        """dst[128, 2, CH'] (bf16) = w^T @ src + bias; src [128, 2, CH'] fm."""
        ncol = dst.shape[2]
        for mt in range(2):
            pp = ps.tile([128, 384], f32, tag="pp", bufs=2)
            for kt in range(2):
                nc.tensor.matmul(pp[:, 0:ncol], w[:, kt, 128 * mt:128 * (mt + 1)],
                                 src[:, kt, :], start=(kt == 0), stop=(kt == 1))
            copy_fns[mt % len(copy_fns)](dst[:, mt, :], pp[:, 0:ncol],
                                         bias[:, mt:mt + 1])
        return dst

    def act_copy_bias(dst, src, biasap):
        nc.scalar.activation(out=dst, in_=src, func=AF.Identity, bias=biasap)

    def dve_copy_bias(dst, src, biasap):
        nc.vector.tensor_scalar(out=dst, in0=src, scalar1=biasap,
                                scalar2=None, op0=ALU.add)

    def pool_copy_bias(dst, src, biasap):
        nc.gpsimd.tensor_scalar(out=dst, in0=src, scalar1=biasap,
                                scalar2=None, op0=ALU.add)

    # ---------------- main chunk loop ----------------
    for c in range(NCH):
        # ---- load + transpose x chunk: xch [128, 2, CH] f32r ----
        xin = p.tile([96, 4, D], f32r, tag="xin")
        nc.sync.dma_start(out=xin.bitcast(f32),
                          in_=x_d.ap()[:, 4 * c:4 * c + 4, :])
        xch = p.tile([128, 2, CH], f32r, tag="xch")
        transpose_in(xch, lambda i: 96 * i, xin,
                     lambda i, kt: xin[:, i, 128 * kt:128 * (kt + 1)],
                     4, [pool_copy, dve_copy])

        # ---- SA projections ----
        qch = p.tile([128, 2, CH], bf16, tag="qch")
        proj_fm(qch, wq, bq, xch, [act_copy_bias, pool_copy_bias])
        kch = p.tile([128, 2, CH], bf16, tag="kch")
        proj_fm(kch, wk, bk, xch, [dve_copy_bias, pool_copy_bias])
        vts = []
        for j in range(4):
            pv = ps.tile([128, 384], f32, tag="pp", bufs=2)
            for kt in range(2):
                nc.tensor.matmul(pv[0:96, 0:D], xch[:, kt, 96 * j:96 * j + 96],
                                 wv[:, kt, :], start=(kt == 0), stop=(kt == 1))
            vt = p.tile([96, D], bf16, tag="vt", bufs=8)
            nc.gpsimd.tensor_copy(vt, pv[0:96, 0:D])
            vts.append(vt)

        # ---- SA attention -> och ----
        och = p.tile([128, 2, CH], bf16, tag="och")
        for j in range(4):
            for g in range(2):
                attn_group(qch, 96 * j, kch, [(96 * j, vts[j])], None,
                           och, g, j, causal=True)

        # ---- SA O-proj + residual -> u ----
        u = p.tile([128, 2, CH], bf16, tag="u")
        for mt in range(2):
            pp = ps.tile([128, 384], f32, tag="pp", bufs=2)
            for kt in range(2):
                nc.tensor.matmul(pp, wo[:, kt, 128 * mt:128 * (mt + 1)],
                                 och[:, kt, :], start=(kt == 0), stop=(kt == 1))
            nc.vector.scalar_tensor_tensor(
                out=u[:, mt, :], in0=pp, scalar=bo2[:, mt:mt + 1],
                in1=xch[:, mt, :].bitcast(f32), op0=ALU.add, op1=ALU.add)
        h1 = fm_layernorm(u, LNP["sa_ln1"], 0, "h1")

        # ---- SA FFN ----
        hh = p.tile([128, FF // 128, CH], bf16, tag="hh")
        for mt in range(FF // 128):
            pp = ps.tile([128, 384], f32, tag="pp", bufs=2)
            for kt in range(2):
                nc.tensor.matmul(pp, w1[:, kt, 128 * mt:128 * (mt + 1)],
                                 h1[:, kt, :], start=(kt == 0), stop=(kt == 1))
            if mt % 2 == 0:
                nc.scalar.activation(out=hh[:, mt, :], in_=pp, func=AF.Relu,
                                     bias=fb1[:, mt:mt + 1])
            else:
                nc.gpsimd.tensor_scalar(out=hh[:, mt, :], in0=pp,
                                        scalar1=fb1[:, mt:mt + 1], scalar2=0.0,
                                        op0=ALU.add, op1=ALU.max)
        u2 = p.tile([128, 2, CH], bf16, tag="u2")
        for mt in range(2):
            pp = ps.tile([128, 384], f32, tag="pp", bufs=2)
            for kt in range(FF // 128):
                nc.tensor.matmul(pp, w2[:, kt, 128 * mt:128 * (mt + 1)],
                                 hh[:, kt, :], start=(kt == 0),
                                 stop=(kt == FF // 128 - 1))
            nc.vector.scalar_tensor_tensor(
                out=u2[:, mt, :], in0=pp, scalar=fb2[:, mt:mt + 1],
                in1=h1[:, mt, :], op0=ALU.add, op1=ALU.add)
        s2 = fm_layernorm(u2, LNP["sa_ln2"], 1, "s2")
        u3 = p.tile([128, 2, CH], bf16, tag="u3")
        nc.gpsimd.tensor_tensor(out=u3, in0=s2, in1=xch.bitcast(f32), op=ALU.add)
        x1 = fm_layernorm(u3, LNP["ln1"], 2, "x1")

        # ---- CA q-projection ----
        qc2 = p.tile([128, 2, CH], bf16, tag="qc2")
        proj_fm(qc2, cwq, cbq, x1, [act_copy_bias, dve_copy_bias])

        # ---- memory load/transpose + K/V projections (2 mchunks) ----
        kc2s, vcs = [], {}
        for mc2 in range(2):
            mch = p.tile([128, 2, CH], f32r, tag="mch", bufs=3)
            for tt in range(2):
                min_ = p.tile([96, 2, D], f32r, tag="min", bufs=4)
                nc.sync.dma_start(
                    out=min_.bitcast(f32),
                    in_=mem_d.ap()[96 * tt:96 * tt + 96,
                                   4 * c + 2 * mc2:4 * c + 2 * mc2 + 2, :])
                transpose_in(mch, lambda i, _tt=tt: 192 * i + 96 * _tt, min_,
                             lambda i, kt, _m=min_: _m[:, i, 128 * kt:128 * (kt + 1)],
                             2, [pool_copy, dve_copy])
            kc2 = p.tile([128, 2, CH], bf16, tag="kc2", bufs=3)
            proj_fm(kc2, cwk, cbk, mch, [dve_copy_bias, pool_copy_bias])
            kc2s.append(kc2)
            for jj in range(2):
                for tt in range(2):
                    seg = 192 * jj + 96 * tt
                    pv = ps.tile([128, 384], f32, tag="pp", bufs=2)
                    for kt in range(2):
                        nc.tensor.matmul(pv[0:96, 0:D],
                                         mch[:, kt, seg:seg + 96],
                                         cwv[:, kt, :], start=(kt == 0),
                                         stop=(kt == 1))
                    vc = p.tile([96, D], bf16, tag="vc", bufs=10)
                    nc.gpsimd.tensor_copy(vc, pv[0:96, 0:D])
                    vcs[(2 * mc2 + jj, tt)] = vc

        # ---- CA attention -> och2 ----
        och2 = p.tile([128, 2, CH], bf16, tag="och2")
        for j in range(4):
            kc2 = kc2s[j // 2]
            k0 = 192 * (j % 2)
            kslices = [(k0, vcs[(j, 0)]), (k0 + 96, vcs[(j, 1)])]
            for g in range(2):
                attn_group(qc2, 96 * j, kc2, kslices, None, och2, g, j,
                           causal=False)

        # ---- CA O-proj + residual ----
        u4 = p.tile([128, 2, CH], bf16, tag="u4")
        for mt in range(2):
            pp = ps.tile([128, 384], f32, tag="pp", bufs=2)
            for kt in range(2):
                nc.tensor.matmul(pp, cwo[:, kt, 128 * mt:128 * (mt + 1)],
                                 och2[:, kt, :], start=(kt == 0), stop=(kt == 1))
            nc.vector.scalar_tensor_tensor(
                out=u4[:, mt, :], in0=pp, scalar=cbo2[:, mt:mt + 1],
                in1=x1[:, mt, :], op0=ALU.add, op1=ALU.add)
        x2 = fm_layernorm(u4, LNP["ln2"], 3, "x2")

        # ---- decoder FFN ----
        hh2 = p.tile([128, FF // 128, CH], bf16, tag="hh2")
        for mt in range(FF // 128):
            pp = ps.tile([128, 384], f32, tag="pp", bufs=2)
            for kt in range(2):
                nc.tensor.matmul(pp, fw1[:, kt, 128 * mt:128 * (mt + 1)],
                                 x2[:, kt, :], start=(kt == 0), stop=(kt == 1))
            if mt % 2 == 0:
                nc.scalar.activation(out=hh2[:, mt, :], in_=pp, func=AF.Relu,
                                     bias=fbb1[:, mt:mt + 1])
            else:
                nc.gpsimd.tensor_scalar(out=hh2[:, mt, :], in0=pp,
                                        scalar1=fbb1[:, mt:mt + 1], scalar2=0.0,
                                        op0=ALU.add, op1=ALU.max)
        u5 = p.tile([128, 2, CH], bf16, tag="u5")
        for mt in range(2):
            pp = ps.tile([128, 384], f32, tag="pp", bufs=2)
            for kt in range(FF // 128):
                nc.tensor.matmul(pp, fw2[:, kt, 128 * mt:128 * (mt + 1)],
                                 hh2[:, kt, :], start=(kt == 0),
                                 stop=(kt == FF // 128 - 1))
            nc.vector.scalar_tensor_tensor(
                out=u5[:, mt, :], in0=pp, scalar=fbb2[:, mt:mt + 1],
                in1=x2[:, mt, :], op0=ALU.add, op1=ALU.add)
        oo = fm_layernorm(u5, LNP["ln3"], 4, "oo", out_dtype=f32r)

        # ---- transpose back + store ----
        tm = p.tile([96, 4, D], f32, tag="tm")
        for j in range(4):
            for kt in range(2):
                tq = ps.tile([128, 384], f32, tag="pp", bufs=2)
                tqv = tq[0:96, 0:128]
                nc.tensor.transpose(
                    tqv.bitcast(f32r),
                    oo[:, kt, 96 * j:96 * j + 96], idr)
                eng = pool_copy if (j + kt) % 2 == 0 else dve_copy
                eng(tm[:, j, 128 * kt:128 * (kt + 1)], tqv)
        nc.sync.dma_start(out=out_d.ap()[:, 4 * c:4 * c + 4, :], in_=tm)
    es.close()


def _make_runner(nc):
    """Cached jitted SPMD runner (avoids per-call retracing of
    run_bass_via_pjrt's fresh closures)."""
    import jax
    import numpy as np
    from jax.sharding import Mesh, PartitionSpec
    from jax.experimental.shard_map import shard_map
    import concourse.mybir as mybir
    from concourse.bass2jax import (_bass_exec_p, install_neuronx_cc_hook,
                                    partition_id_tensor)

    install_neuronx_cc_hook()
    partition_name = (nc.partition_id_tensor.name
                      if nc.partition_id_tensor else None)
    in_names, out_names, out_avals, zero_outs = [], [], [], []
    for alloc in nc.m.functions[0].allocations:
        if not isinstance(alloc, mybir.MemoryLocationSet):
            continue
        name = alloc.memorylocations[0].name
        if alloc.kind == "ExternalInput":
            if name != partition_name:
                in_names.append(name)
        elif alloc.kind == "ExternalOutput":
            shape = tuple(alloc.tensor_shape)
            dtype = mybir.dt.np(alloc.dtype)
            out_names.append(name)
            out_avals.append(jax.core.ShapedArray(shape, dtype))
            zero_outs.append(np.zeros(shape, dtype))
    n_params = len(in_names)
    all_names = list(in_names) + list(out_names)
    if partition_name is not None:
        all_names.append(partition_name)
    donate = tuple(range(n_params, n_params + len(out_names)))

    def _body(*args):
        operands = list(args)
        if partition_name is not None:
            operands.append(partition_id_tensor())
        return tuple(_bass_exec_p.bind(
            *operands, out_avals=tuple(out_avals), in_names=tuple(all_names),
            out_names=tuple(out_names), lowering_input_output_aliases=(),
            sim_require_finite=True, sim_require_nnan=True, nc=nc))

    devices = jax.devices()[:B]
    mesh = Mesh(np.asarray(devices), ("core",))
    in_specs = (PartitionSpec("core"),) * (n_params + len(out_names))
    out_specs = (PartitionSpec("core"),) * len(out_names)
    sharded = jax.jit(shard_map(_body, mesh=mesh, in_specs=in_specs,
                                out_specs=out_specs, check_rep=False),
                      donate_argnums=donate, keep_unused=True)

    def run(in_maps):
        concat_in = [np.concatenate([np.asarray(in_maps[c][nm])
                                     for c in range(B)], axis=0)
                     for nm in in_names]
        concat_zeros = [np.zeros((B * z.shape[0], *z.shape[1:]), z.dtype)
                        for z in zero_outs]
        out_arrs = sharded(*concat_in, *concat_zeros)
        oidx = out_names.index("out")
        a = np.asarray(out_arrs[oidx])
        return a.reshape(B, *out_avals[oidx].shape)

    return run


def kernel(**inputs):
    if "nc" not in _cached:
        _cached["nc"] = _build()
        _cached["run"] = _make_runner(_cached["nc"])
    inp = {k: np.asarray(v, dtype=np.float32) for k, v in inputs.items()}
    shared = {k: v for k, v in inp.items() if k not in ("x", "memory")}
    in_maps = []
    for c in range(B):
        m = dict(shared)
        m["x"] = np.ascontiguousarray(inp["x"][c])
        m["memory"] = np.ascontiguousarray(inp["memory"][c])
        in_maps.append(m)
    out = _cached["run"](in_maps)
    return np.ascontiguousarray(out).astype(np.float32)


# revision 24
# speedup vs baseline: 1.0915x; 1.0368x over previous
"""Trainium2 Bass kernel for nn_DecoderLayer (temporal self-attn decoder layer).

Sharding: data-parallel over batch B=8, one batch per NeuronCore. Weights are
replicated; each core runs an identical program on its batch slice.

Fully-fused single-pass design: one loop over 16 chunks (4 hw columns each,
CH=384 tokens). All intermediates stay in SBUF (no DRAM scratch). Activations
are feature-major X^T [D=256 -> 2 slabs x 128 partitions, tokens]; attention
works on per-head partition slices (heads = 32-partition blocks), causal mask
is folded into the scores via a PE bias-add matmul, softmax normalization is
folded into P before the AV matmuls. LayerNorm rstd uses exp(-0.5*ln(var)) so
the Activation engine only ever needs one table set (ln/exp/relu/square/copy).
"""
import numpy as np

D, NH, HD, FF = 256, 8, 32, 1024
B, T_OUT, T_IN, HW = 8, 96, 192, 64
CH = 384                 # tokens per chunk = 4 hw (x side), 2 hw (memory side)
NCH = (T_OUT * HW) // CH # 16
SCALE = float(1.0 / np.sqrt(HD))
LN_EPS_ARG = float(D) * float(D) * 1e-5   # bias for ln(D^2 * var)
LN_LOGD = float(np.log(float(D)))         # exp(-0.5*ln(arg) + ln(D)) = D/sqrt(arg)
MASKNEG = -100.0

_cached = {}


def _build():
    import concourse.bass as bass
    import concourse.mybir as mybir
    import concourse.tile as tile
    from concourse import bacc
    from concourse.masks import make_identity

    f32 = mybir.dt.float32
    f32r = mybir.dt.float32r
    bf16 = mybir.dt.bfloat16
    AF = mybir.ActivationFunctionType
    ALU = mybir.AluOpType

    nc = bacc.Bacc("TRN2", target_bir_lowering=False, debug=False)

    x_d = nc.dram_tensor("x", [T_OUT, HW, D], f32, kind="ExternalInput")
    mem_d = nc.dram_tensor("memory", [T_IN, HW, D], f32, kind="ExternalInput")
    out_d = nc.dram_tensor("out", [T_OUT, HW, D], f32, kind="ExternalOutput")

    WNAMES = [
        ("sa_wq", D, D), ("sa_wk", D, D), ("sa_wv", D, D), ("sa_wo", D, D),
        ("sa_ff_w1", D, FF), ("sa_ff_w2", FF, D),
        ("ca_wq", D, D), ("ca_wk", D, D), ("ca_wv", D, D), ("ca_wo", D, D),
        ("ff_w1", D, FF), ("ff_w2", FF, D),
    ]
    BNAMES = [("sa_bq", D), ("sa_bk", D), ("sa_bv", D), ("sa_bo", D),
              ("sa_ff_b1", FF), ("sa_ff_b2", D),
              ("ca_bq", D), ("ca_bk", D), ("ca_bv", D), ("ca_bo", D),
              ("ff_b1", FF), ("ff_b2", D),
              ("sa_ln1_g", D), ("sa_ln1_b", D), ("sa_ln2_g", D), ("sa_ln2_b", D),
              ("ln1_g", D), ("ln1_b", D), ("ln2_g", D), ("ln2_b", D),
              ("ln3_g", D), ("ln3_b", D)]
    wd = {n: nc.dram_tensor(n, [ki, ko], f32, kind="ExternalInput")
          for n, ki, ko in WNAMES}
    bd = {n: nc.dram_tensor(n, [k], f32, kind="ExternalInput") for n, k in BNAMES}

    with tile.TileContext(nc) as tc:
        _emit(nc, tc, bass, mybir, tile, make_identity, f32, f32r, bf16,
              AF, ALU, x_d, mem_d, out_d, wd, bd)
    nc.compile()
    return nc


def _emit(nc, tc, bass, mybir, tile, make_identity, f32, f32r, bf16,
          AF, ALU, x_d, mem_d, out_d, wd, bd):
    from contextlib import ExitStack

    es = ExitStack()
    cn = es.enter_context(tc.tile_pool(name="consts", bufs=1))
    pw = es.enter_context(tc.tile_pool(name="wts", bufs=1))
    pstg_cm = tc.tile_pool(name="wstage", bufs=1)
    pstg = pstg_cm.__enter__()
    ps = es.enter_context(tc.tile_pool(name="ps", bufs=2, space="PSUM"))

    # ---------------- constants ----------------
    idf = cn.tile([128, 128], f32, tag="idf")
    make_identity(nc, idf)
    idr = cn.tile([128, 128], f32r, tag="idr")     # f32r identity for transposes
    nc.gpsimd.tensor_copy(idr, idf)
    ones128b = cn.tile([128, 1], bf16, tag="ones128b")
    nc.vector.memset(ones128b, 1.0)
    ones96b = cn.tile([T_OUT, 1], bf16, tag="ones96b")
    nc.vector.memset(ones96b, 1.0)
    epsarg = cn.tile([1, 1], f32, tag="epsarg")
    nc.vector.memset(epsarg, LN_EPS_ARG)
    logd = cn.tile([1, 1], f32, tag="logd")
    nc.vector.memset(logd, LN_LOGD)
    # causal mask lhsT: L[p, m] = MASKNEG if m > p else 0  (bf16)
    maskf = cn.tile([T_OUT, T_OUT], f32, tag="maskf")
    nc.vector.memset(maskf, MASKNEG)
    nc.gpsimd.affine_select(out=maskf, in_=maskf, compare_op=ALU.is_ge,
                            fill=0.0, base=-1, pattern=[[1, T_OUT]],
                            channel_multiplier=-1)
    maskb = cn.tile([T_OUT, T_OUT], bf16, tag="maskb")
    nc.vector.tensor_copy(maskb, maskf)
    # identity replicated over the 4-head dim: [96, 4, 96] bf16 (materialized)
    idrep = cn.tile([T_OUT, 4, T_OUT], bf16, tag="idrep")
    idsrc = idf[0:T_OUT, 0:T_OUT]
    nc.gpsimd.tensor_copy(
        idrep, bass.AP(tensor=idsrc.tensor, offset=idsrc.offset,
                       ap=[idsrc.ap[0], [0, 4], idsrc.ap[1]]))
    # head-block select: blkb[k, p] = 1 iff k == 32*(p//32)
    blkb = cn.tile([128, 128], bf16, tag="blkb")
    nc.vector.memset(blkb, 0.0)
    for a in range(4):
        nc.vector.memset(blkb[32 * a:32 * a + 1, 32 * a:32 * a + 32], 1.0)
    ones96x128 = cn.tile([T_OUT, 128], bf16, tag="ones96x128")
    nc.vector.memset(ones96x128, 1.0)

    # ---------------- weights ----------------
    def load_w(name, ki, ko, dtype, tag, scale=None):
        """[ki, ko] f32 DRAM -> [128, ki//128, ko] SBUF tile of dtype."""
        src = wd[name].ap().rearrange("(kt p) n -> p kt n", p=128)
        if dtype == f32r:
            wt = pw.tile([128, ki // 128, ko], f32r, tag=tag)
            nc.sync.dma_start(out=wt.bitcast(f32), in_=src)
            if scale is not None:
                nc.gpsimd.tensor_scalar(out=wt.bitcast(f32), in0=wt.bitcast(f32),
                                        scalar1=scale, scalar2=None, op0=ALU.mult)
            return wt
        stg = pstg.tile([128, ki // 128, ko], f32, tag=tag + "_stg")
        nc.sync.dma_start(out=stg, in_=src)
        wt = pw.tile([128, ki // 128, ko], bf16, tag=tag)
        if scale is not None:
            nc.gpsimd.tensor_scalar(out=wt, in0=stg, scalar1=scale,
                                    scalar2=None, op0=ALU.mult)
        else:
            nc.gpsimd.tensor_copy(wt, stg)
        return wt

    def load_b(name, k, tag, scale=None):
        bt = pw.tile([128, k // 128], f32, tag=tag)
        nc.sync.dma_start(out=bt, in_=bd[name].ap().rearrange(
            "(kt p) -> p kt", p=128))
        if scale is not None:
            nc.gpsimd.tensor_scalar(out=bt, in0=bt, scalar1=scale,
                                    scalar2=None, op0=ALU.mult)
        return bt

    wq = load_w("sa_wq", D, D, f32r, "wq", scale=SCALE)
    wk = load_w("sa_wk", D, D, f32r, "wk")
    wv = load_w("sa_wv", D, D, f32r, "wv")
    wo = load_w("sa_wo", D, D, bf16, "wo")
    w1 = load_w("sa_ff_w1", D, FF, bf16, "w1")
    w2 = load_w("sa_ff_w2", FF, D, bf16, "w2")
    cwq = load_w("ca_wq", D, D, bf16, "cwq", scale=SCALE)
    cwk = load_w("ca_wk", D, D, f32r, "cwk")
    cwv = load_w("ca_wv", D, D, f32r, "cwv")
    cwo = load_w("ca_wo", D, D, bf16, "cwo")
    fw1 = load_w("ff_w1", D, FF, bf16, "fw1")
    fw2 = load_w("ff_w2", FF, D, bf16, "fw2")

    bq = load_b("sa_bq", D, "bq", scale=SCALE)
    bk = load_b("sa_bk", D, "bk")
    fb1 = load_b("sa_ff_b1", FF, "fb1")
    fb2 = load_b("sa_ff_b2", D, "fb2")
    cbq = load_b("ca_bq", D, "cbq", scale=SCALE)
    cbk = load_b("ca_bk", D, "cbk")
    fbb1 = load_b("ff_b1", FF, "fbb1")
    fbb2 = load_b("ff_b2", D, "fbb2")

    # fused O-proj biases: bo2 = Wo^T bv + bo (per output feature)
    def oproj_bias(wox, bvname, boname, tag):
        bvf = load_b(bvname, D, tag + "v")
        bof = load_b(boname, D, tag + "o")
        bvr = pw.tile([128, 2, 128], bf16, tag=tag + "r")
        for kt in range(2):
            nc.vector.tensor_copy(bvr[:, kt, :],
                                  bvf[:, kt:kt + 1].to_broadcast([128, 128]))
        bo2 = pw.tile([128, 2], f32, tag=tag + "2")
        for mt in range(2):
            bps = ps.tile([128, 384], f32, tag="pp", bufs=3)
            for kt in range(2):
                nc.tensor.matmul(bps[:, 0:128], wox[:, kt, 128 * mt:128 * (mt + 1)],
                                 bvr[:, kt, :], start=(kt == 0), stop=(kt == 1))
            nc.vector.tensor_tensor(out=bo2[:, mt:mt + 1], in0=bps[:, 0:1],
                                    in1=bof[:, mt:mt + 1], op=ALU.add)
        return bo2

    bo2 = oproj_bias(wo, "sa_bv", "sa_bo", "bo")
    cbo2 = oproj_bias(cwo, "ca_bv", "ca_bo", "cbo")

    # layernorm params: g, b, and gD = -g/D
    def ln_params(gname, bname, tag):
        g = load_b(gname, D, tag + "g")
        b = load_b(bname, D, tag + "b")
        gD = pw.tile([128, 2], f32, tag=tag + "gD")
        nc.gpsimd.tensor_scalar(out=gD, in0=g, scalar1=-1.0 / D, scalar2=None,
                                op0=ALU.mult)
        return g, b, gD

    LNP = {k: ln_params(k + "_g", k + "_b", k) for k in
           ("sa_ln1", "sa_ln2", "ln1", "ln2", "ln3")}
    pstg_cm.__exit__(None, None, None)
    p = es.enter_context(tc.tile_pool(name="work", bufs=2))

    # ---------------- helpers ----------------
    def fm_layernorm(u, lnp, idx, tag, out_dtype=bf16):
        """u: [128, 2, CH] bf16 SBUF -> normalized tile (out_dtype)."""
        g, b, gD = lnp
        usq = p.tile([128, 2, CH], bf16, tag="usq")
        nc.scalar.activation(out=usq, in_=u, func=AF.Square)
        ps_st = ps.tile([T_OUT, 4, T_OUT], f32, tag="st", bufs=3)
        ps_qt = ps.tile([T_OUT, 4, T_OUT], f32, tag="st", bufs=3)
        ps_s = ps_st.rearrange("p a b -> p (a b)")[0:1, 0:CH]
        ps_q = ps_qt.rearrange("p a b -> p (a b)")[0:1, 0:CH]
        for kt in range(2):
            nc.tensor.matmul(ps_s, ones128b, u[:, kt, :], start=(kt == 0),
                             stop=(kt == 1))
        for kt in range(2):
            nc.tensor.matmul(ps_q, ones128b, usq[:, kt, :], start=(kt == 0),
                             stop=(kt == 1))
        msq = p.tile([1, CH], f32, tag="lnmsq")
        nc.gpsimd.tensor_tensor(out=msq, in0=ps_s, in1=ps_s, op=ALU.mult)
        arg = p.tile([1, CH], f32, tag="lnarg")
        nc.gpsimd.scalar_tensor_tensor(out=arg, in0=ps_q, scalar=float(D),
                                       in1=msq, op0=ALU.mult, op1=ALU.subtract)
        # rstd = D * arg^-0.5 = 1/sqrt(var)
        rc = p.tile([1, 2, CH], bf16, tag="lnrc")
        with nc.allow_low_precision(reason="rstd in bf16 is fine"):
            nc.vector.tensor_scalar(out=rc[:, 0, :], in0=arg, scalar1=-0.5,
                                    scalar2=float(D), op0=ALU.pow, op1=ALU.mult)
        nc.gpsimd.tensor_tensor(out=rc[:, 1, :], in0=ps_s, in1=rc[:, 0, :],
                                op=ALU.mult)
        rcb = p.tile([128, 2, CH], bf16, tag="lnrcb")
        nc.gpsimd.partition_broadcast(
            rcb.rearrange("p a b -> p (a b)"), rc.rearrange("p a b -> p (a b)"))
        o = p.tile([128, 2, CH], out_dtype, tag="lno" + tag)
        for kt in range(2):
            ft = p.tile([128, CH], bf16, tag="lnf")
            nc.vector.tensor_scalar(out=ft, in0=rcb[:, 1, :],
                                    scalar1=gD[:, kt:kt + 1],
                                    scalar2=b[:, kt:kt + 1],
                                    op0=ALU.mult, op1=ALU.add)
            t1 = p.tile([128, CH], bf16, tag="lnt1")
            if kt == 0:
                nc.vector.tensor_tensor(out=t1, in0=u[:, kt, :],
                                        in1=rcb[:, 0, :], op=ALU.mult)
                nc.vector.scalar_tensor_tensor(out=o[:, kt, :], in0=t1,
                                               scalar=g[:, kt:kt + 1], in1=ft,
                                               op0=ALU.mult, op1=ALU.add)
            else:
                nc.gpsimd.tensor_tensor(out=t1, in0=u[:, kt, :],
                                        in1=rcb[:, 0, :], op=ALU.mult)
                nc.gpsimd.scalar_tensor_tensor(out=o[:, kt, :], in0=t1,
                                               scalar=g[:, kt:kt + 1], in1=ft,
                                               op0=ALU.mult, op1=ALU.add)
        return o

    def transpose_in(dst, dst_cols, src_slices):
        """PE-transpose [96, 128] slabs into dst[:, kt, col:col+96] (fm).
        Batches all slabs of one kt into a single [128, 384] psum tile with
        one copy out. dst_cols/src_slices indexed by slab i."""
        n = len(dst_cols)
        for kt in range(2):
            tb = ps.tile([128, 384], f32r, tag="pp", bufs=3)
            for i in range(n):
                nc.tensor.transpose(tb[:, 96 * i:96 * i + 96],
                                    src_slices(i, kt), idr[0:96, 0:96])
            if n == 4 and all(dst_cols[i] == 96 * i for i in range(4)):
                eng = pool_copy if kt == 0 else dve_copy
                eng(dst[:, kt, :], tb)
            else:
                for i in range(n):
                    eng = pool_copy if (i + kt) % 2 == 0 else dve_copy
                    eng(dst[:, kt, dst_cols[i]:dst_cols[i] + 96],
                        tb[:, 96 * i:96 * i + 96])

    def pool_copy(dst, srctile):
        nc.gpsimd.tensor_copy(dst, srctile)

    def dve_copy(dst, srctile):
        nc.vector.tensor_copy(dst, srctile)

    # attention core for one (j, g) group of 4 heads
    def attn_group(qsrc, q0, ksrc, kslices, vparts, och, g, j, causal):
        """qsrc/ksrc: [128, 2, CH'] bf16 fm tiles; kslices: list of
        (col0, vtile) for each 96-token k block; och: [128, 2, CH] bf16."""
        nk = len(kslices)
        pts = []
        for tt in range(nk):
            k0, _ = kslices[tt]
            st = ps.tile([T_OUT, 4, T_OUT], f32, tag="st", bufs=3)
            for hp in range(4):
                nc.tensor.matmul(st[:, hp, :],
                                 ksrc[32 * hp:32 * hp + 32, g, k0:k0 + 96],
                                 qsrc[32 * hp:32 * hp + 32, g, q0:q0 + 96],
                                 start=True, stop=not causal,
                                 skip_group_check=True,
                                 tile_position=(32 * hp, 0))
            if causal:
                nc.tensor.matmul(st, maskb, idrep, start=False, stop=True,
                                 skip_group_check=True)
            pt = p.tile([T_OUT, 4, T_OUT], bf16, tag=f"pt{tt}", bufs=3)
            nc.scalar.activation(out=pt, in_=st, func=AF.Exp)
            pts.append(pt)
        # per-head denominators as 1-row stripes of a [128, 96] psum tile;
        # a first full-tile matmul fills the unused partitions with benign
        # positive values so the whole-tile reciprocal stays finite.
        dnvt = ps.tile([128, 96], f32r, tag="tp", bufs=2)
        dnv = dnvt.bitcast(f32)
        nc.tensor.matmul(dnv, ones96x128, pts[0][:, 0, :], start=True,
                         stop=False, skip_group_check=True)
        for hp in range(4):
            for tt in range(nk):
                nc.tensor.matmul(dnv[32 * hp:32 * hp + 1, :], ones96b,
                                 pts[tt][:, hp, :], start=(tt == 0),
                                 stop=(tt == nk - 1), skip_group_check=True,
                                 tile_position=(0, 32 * hp))
        recn = p.tile([128, 96], bf16, tag="recn", bufs=3)
        with nc.allow_low_precision(reason="softmax denom in bf16 is fine"):
            nc.vector.reciprocal(out=recn, in_=dnv)
        recb = dnv
        nc.tensor.matmul(recb, blkb, recn, start=True, stop=True,
                         skip_group_check=True)
        avt = ps.tile([128, 96], f32r, tag="tp", bufs=2)
        av = avt.bitcast(f32)
        for hp in range(4):
            h = g * 4 + hp
            for tt in range(nk):
                _, vt = kslices[tt]
                nc.tensor.matmul(av[32 * hp:32 * hp + 32, :],
                                 vt[:, 32 * h:32 * h + 32], pts[tt][:, hp, :],
                                 start=(tt == 0), stop=(tt == nk - 1),
                                 skip_group_check=True,
                                 tile_position=(0, 32 * hp))
        if (j + g) % 2 == 0:
            nc.gpsimd.tensor_tensor(out=och[:, g, 96 * j:96 * j + 96],
                                    in0=av, in1=recb, op=ALU.mult)
        else:
            nc.vector.tensor_tensor(out=och[:, g, 96 * j:96 * j + 96],
                                    in0=av, in1=recb, op=ALU.mult)

    def proj_fm(dst, w, bias, src, copy_fns):
        """dst[128, 2, CH'] (bf16) = w^T @ src + bias; src [128, 2, CH'] fm."""
        ncol = dst.shape[2]
        for mt in range(2):
            pp = ps.tile([128, 384], f32, tag="pp", bufs=3)
            for kt in range(2):
                nc.tensor.matmul(pp[:, 0:ncol], w[:, kt, 128 * mt:128 * (mt + 1)],
                                 src[:, kt, :], start=(kt == 0), stop=(kt == 1))
            copy_fns[mt % len(copy_fns)](dst[:, mt, :], pp[:, 0:ncol],
                                         bias[:, mt:mt + 1])
        return dst

    def act_copy_bias(dst, src, biasap):
        nc.scalar.activation(out=dst, in_=src, func=AF.Identity, bias=biasap)

    def dve_copy_bias(dst, src, biasap):
        nc.vector.tensor_scalar(out=dst, in0=src, scalar1=biasap,
                                scalar2=None, op0=ALU.add)

    def pool_copy_bias(dst, src, biasap):
        nc.gpsimd.tensor_scalar(out=dst, in0=src, scalar1=biasap,
                                scalar2=None, op0=ALU.add)

    # ---------------- main chunk loop ----------------
    for c in range(NCH):
        # ---- load + transpose x chunk: xch [128, 2, CH] f32r ----
        xin = p.tile([96, 4, D], f32r, tag="xin")
        nc.sync.dma_start(out=xin.bitcast(f32),
                          in_=x_d.ap()[:, 4 * c:4 * c + 4, :])
        xch = p.tile([128, 2, CH], f32r, tag="xch")
        transpose_in(xch, [0, 96, 192, 288],
                     lambda i, kt: xin[:, i, 128 * kt:128 * (kt + 1)])

        # ---- SA projections ----
        qch = p.tile([128, 2, CH], bf16, tag="qch")
        proj_fm(qch, wq, bq, xch, [act_copy_bias, pool_copy_bias])
        kch = p.tile([128, 2, CH], bf16, tag="kch")
        proj_fm(kch, wk, bk, xch, [dve_copy_bias, pool_copy_bias])
        vts = []
        for j in range(4):
            pv = ps.tile([128, 384], f32, tag="pp", bufs=3)
            for kt in range(2):
                nc.tensor.matmul(pv[0:96, 0:D], xch[:, kt, 96 * j:96 * j + 96],
                                 wv[:, kt, :], start=(kt == 0), stop=(kt == 1))
            vt = p.tile([96, D], bf16, tag="vt", bufs=8)
            nc.gpsimd.tensor_copy(vt, pv[0:96, 0:D])
            vts.append(vt)

        # ---- SA attention -> och ----
        och = p.tile([128, 2, CH], bf16, tag="och")
        for j in range(4):
            for g in range(2):
                attn_group(qch, 96 * j, kch, [(96 * j, vts[j])], None,
                           och, g, j, causal=True)

        # ---- SA O-proj + residual -> u ----
        u = p.tile([128, 2, CH], bf16, tag="u")
        for mt in range(2):
            pp = ps.tile([128, 384], f32, tag="pp", bufs=3)
            for kt in range(2):
                nc.tensor.matmul(pp, wo[:, kt, 128 * mt:128 * (mt + 1)],
                                 och[:, kt, :], start=(kt == 0), stop=(kt == 1))
            nc.vector.scalar_tensor_tensor(
                out=u[:, mt, :], in0=pp, scalar=bo2[:, mt:mt + 1],
                in1=xch[:, mt, :].bitcast(f32), op0=ALU.add, op1=ALU.add)
        h1 = fm_layernorm(u, LNP["sa_ln1"], 0, "h1")

        # ---- SA FFN ----
        hh = p.tile([128, FF // 128, CH], bf16, tag="hh")
        for mt in range(FF // 128):
            pp = ps.tile([128, 384], f32, tag="pp", bufs=3)
            for kt in range(2):
                nc.tensor.matmul(pp, w1[:, kt, 128 * mt:128 * (mt + 1)],
                                 h1[:, kt, :], start=(kt == 0), stop=(kt == 1))
            if mt % 2 == 0:
                nc.scalar.activation(out=hh[:, mt, :], in_=pp, func=AF.Relu,
                                     bias=fb1[:, mt:mt + 1])
            else:
                nc.gpsimd.tensor_scalar(out=hh[:, mt, :], in0=pp,
                                        scalar1=fb1[:, mt:mt + 1], scalar2=0.0,
                                        op0=ALU.add, op1=ALU.max)
        u2 = p.tile([128, 2, CH], bf16, tag="u2")
        for mt in range(2):
            pp = ps.tile([128, 384], f32, tag="pp", bufs=3)
            for kt in range(FF // 128):
                nc.tensor.matmul(pp, w2[:, kt, 128 * mt:128 * (mt + 1)],
                                 hh[:, kt, :], start=(kt == 0),
                                 stop=(kt == FF // 128 - 1))
            nc.vector.scalar_tensor_tensor(
                out=u2[:, mt, :], in0=pp, scalar=fb2[:, mt:mt + 1],
                in1=h1[:, mt, :], op0=ALU.add, op1=ALU.add)
        s2 = fm_layernorm(u2, LNP["sa_ln2"], 1, "s2")
        u3 = p.tile([128, 2, CH], bf16, tag="u3")
        nc.gpsimd.tensor_tensor(out=u3, in0=s2, in1=xch.bitcast(f32), op=ALU.add)
        x1 = fm_layernorm(u3, LNP["ln1"], 2, "x1")

        # ---- CA q-projection ----
        qc2 = p.tile([128, 2, CH], bf16, tag="qc2")
        proj_fm(qc2, cwq, cbq, x1, [act_copy_bias, dve_copy_bias])

        # ---- memory load/transpose + K/V projections (2 mchunks) ----
        kc2s, vcs = [], {}
        for mc2 in range(2):
            mch = p.tile([128, 2, CH], f32r, tag="mch", bufs=3)
            mins = []
            for tt in range(2):
                min_ = p.tile([96, 2, D], f32r, tag="min", bufs=4)
                nc.sync.dma_start(
                    out=min_.bitcast(f32),
                    in_=mem_d.ap()[96 * tt:96 * tt + 96,
                                   4 * c + 2 * mc2:4 * c + 2 * mc2 + 2, :])
                mins.append(min_)
            # slab order i = 2*j2 + tt -> dst col 96*i = 192*j2 + 96*tt
            transpose_in(mch, [0, 96, 192, 288],
                         lambda i, kt: mins[i % 2][:, i // 2,
                                                   128 * kt:128 * (kt + 1)])
            kc2 = p.tile([128, 2, CH], bf16, tag="kc2", bufs=3)
            proj_fm(kc2, cwk, cbk, mch, [dve_copy_bias, pool_copy_bias])
            kc2s.append(kc2)
            for jj in range(2):
                for tt in range(2):
                    seg = 192 * jj + 96 * tt
                    pv = ps.tile([128, 384], f32, tag="pp", bufs=3)
                    for kt in range(2):
                        nc.tensor.matmul(pv[0:96, 0:D],
                                         mch[:, kt, seg:seg + 96],
                                         cwv[:, kt, :], start=(kt == 0),
                                         stop=(kt == 1))
                    vc = p.tile([96, D], bf16, tag="vc", bufs=10)
                    nc.gpsimd.tensor_copy(vc, pv[0:96, 0:D])
                    vcs[(2 * mc2 + jj, tt)] = vc

        # ---- CA attention -> och2 ----
        och2 = p.tile([128, 2, CH], bf16, tag="och2")
        for j in range(4):
            kc2 = kc2s[j // 2]
            k0 = 192 * (j % 2)
            kslices = [(k0, vcs[(j, 0)]), (k0 + 96, vcs[(j, 1)])]
            for g in range(2):
                attn_group(qc2, 96 * j, kc2, kslices, None, och2, g, j,
                           causal=False)

        # ---- CA O-proj + residual ----
        u4 = p.tile([128, 2, CH], bf16, tag="u4")
        for mt in range(2):
            pp = ps.tile([128, 384], f32, tag="pp", bufs=3)
            for kt in range(2):
                nc.tensor.matmul(pp, cwo[:, kt, 128 * mt:128 * (mt + 1)],
                                 och2[:, kt, :], start=(kt == 0), stop=(kt == 1))
            nc.vector.scalar_tensor_tensor(
                out=u4[:, mt, :], in0=pp, scalar=cbo2[:, mt:mt + 1],
                in1=x1[:, mt, :], op0=ALU.add, op1=ALU.add)
        x2 = fm_layernorm(u4, LNP["ln2"], 3, "x2")

        # ---- decoder FFN ----
        hh2 = p.tile([128, FF // 128, CH], bf16, tag="hh2")
        for mt in range(FF // 128):
            pp = ps.tile([128, 384], f32, tag="pp", bufs=3)
            for kt in range(2):
                nc.tensor.matmul(pp, fw1[:, kt, 128 * mt:128 * (mt + 1)],
                                 x2[:, kt, :], start=(kt == 0), stop=(kt == 1))
            if mt % 2 == 0:
                nc.scalar.activation(out=hh2[:, mt, :], in_=pp, func=AF.Relu,
                                     bias=fbb1[:, mt:mt + 1])
            else:
                nc.gpsimd.tensor_scalar(out=hh2[:, mt, :], in0=pp,
                                        scalar1=fbb1[:, mt:mt + 1], scalar2=0.0,
                                        op0=ALU.add, op1=ALU.max)
        u5 = p.tile([128, 2, CH], bf16, tag="u5")
        for mt in range(2):
            pp = ps.tile([128, 384], f32, tag="pp", bufs=3)
            for kt in range(FF // 128):
                nc.tensor.matmul(pp, fw2[:, kt, 128 * mt:128 * (mt + 1)],
                                 hh2[:, kt, :], start=(kt == 0),
                                 stop=(kt == FF // 128 - 1))
            nc.vector.scalar_tensor_tensor(
                out=u5[:, mt, :], in0=pp, scalar=fbb2[:, mt:mt + 1],
                in1=x2[:, mt, :], op0=ALU.add, op1=ALU.add)
        oo = fm_layernorm(u5, LNP["ln3"], 4, "oo", out_dtype=f32r)

        # ---- transpose back + store ----
        tm = p.tile([96, 4, D], f32, tag="tm")
        for j in range(4):
            tq = ps.tile([128, 384], f32, tag="pp", bufs=3)
            for kt in range(2):
                nc.tensor.transpose(
                    tq[0:96, 128 * kt:128 * (kt + 1)].bitcast(f32r),
                    oo[:, kt, 96 * j:96 * j + 96], idr)
            eng = pool_copy if j % 2 == 0 else dve_copy
            eng(tm[:, j, :], tq[0:96, 0:256])
        nc.sync.dma_start(out=out_d.ap()[:, 4 * c:4 * c + 4, :], in_=tm)
    es.close()


def _make_runner(nc):
    """Cached jitted SPMD runner (avoids per-call retracing of
    run_bass_via_pjrt's fresh closures)."""
    import jax
    import numpy as np
    from jax.sharding import Mesh, PartitionSpec
    from jax.experimental.shard_map import shard_map
    import concourse.mybir as mybir
    from concourse.bass2jax import (_bass_exec_p, install_neuronx_cc_hook,
                                    partition_id_tensor)

    install_neuronx_cc_hook()
    partition_name = (nc.partition_id_tensor.name
                      if nc.partition_id_tensor else None)
    in_names, out_names, out_avals, zero_outs = [], [], [], []
    for alloc in nc.m.functions[0].allocations:
        if not isinstance(alloc, mybir.MemoryLocationSet):
            continue
        name = alloc.memorylocations[0].name
        if alloc.kind == "ExternalInput":
            if name != partition_name:
                in_names.append(name)
        elif alloc.kind == "ExternalOutput":
            shape = tuple(alloc.tensor_shape)
            dtype = mybir.dt.np(alloc.dtype)
            out_names.append(name)
            out_avals.append(jax.core.ShapedArray(shape, dtype))
            zero_outs.append(np.zeros(shape, dtype))
    n_params = len(in_names)
    all_names = list(in_names) + list(out_names)
    if partition_name is not None:
        all_names.append(partition_name)
    donate = tuple(range(n_params, n_params + len(out_names)))

    def _body(*args):
        operands = list(args)
        if partition_name is not None:
            operands.append(partition_id_tensor())
        return tuple(_bass_exec_p.bind(
            *operands, out_avals=tuple(out_avals), in_names=tuple(all_names),
            out_names=tuple(out_names), lowering_input_output_aliases=(),
            sim_require_finite=True, sim_require_nnan=True, nc=nc))

    devices = jax.devices()[:B]
    mesh = Mesh(np.asarray(devices), ("core",))
    in_specs = (PartitionSpec("core"),) * (n_params + len(out_names))
    out_specs = (PartitionSpec("core"),) * len(out_names)
    sharded = jax.jit(shard_map(_body, mesh=mesh, in_specs=in_specs,
                                out_specs=out_specs, check_rep=False),
                      donate_argnums=donate, keep_unused=True)

    def run(in_maps):
        concat_in = [np.concatenate([np.asarray(in_maps[c][nm])
                                     for c in range(B)], axis=0)
                     for nm in in_names]
        concat_zeros = [np.zeros((B * z.shape[0], *z.shape[1:]), z.dtype)
                        for z in zero_outs]
        out_arrs = sharded(*concat_in, *concat_zeros)
        oidx = out_names.index("out")
        a = np.asarray(out_arrs[oidx])
        return a.reshape(B, *out_avals[oidx].shape)

    return run


def kernel(**inputs):
    if "nc" not in _cached:
        _cached["nc"] = _build()
        _cached["run"] = _make_runner(_cached["nc"])
    inp = {k: np.asarray(v, dtype=np.float32) for k, v in inputs.items()}
    shared = {k: v for k, v in inp.items() if k not in ("x", "memory")}
    in_maps = []
    for c in range(B):
        m = dict(shared)
        m["x"] = np.ascontiguousarray(inp["x"][c])
        m["memory"] = np.ascontiguousarray(inp["memory"][c])
        in_maps.append(m)
    out = _cached["run"](in_maps)
    return np.ascontiguousarray(out).astype(np.float32)
